# revision 19
# baseline (speedup 1.0000x reference)
"""Trainium2 Bass kernel for a 2-layer GAT (graph attention network).

Strategy (8 NeuronCores, SPMD single program):
  - Nodes are partitioned contiguously across the 8 cores by destination;
    within each core the owned nodes are sorted by in-degree (descending)
    and laid out in chunks of 128 (padded-CSR layout).  All cores share one
    static per-chunk slot schedule, so the traced program is SPMD-uniform.
  - Gathers use the GPSIMD dma_gather custom instruction (mlp ucode
    library): one instruction fetches thousands of 256-byte rows.  Row
    indices are int16, so the node table is addressed through THREE
    overlapping windows: A = rows [0, 32768) (contiguous), B = even rows
    (stride 512B), C = odd rows.  Each dst row's source slots are assigned
    to windows at host prep time (low positions prefer A; high positions
    are forced to B/C by parity), which keeps the padded slot count equal
    to the single-window schedule.
  - The layer-1 node table T1[pos] = h (128 ch, bf16, 256B rows).  The
    per-edge attention terms alpha_src are recomputed on-device from the
    gathered h rows (DVE multiply + per-head strided reduce) since they
    are linear in h.  alpha_dst for owned rows comes from a tiny per-core
    matmul of the owned x columns against the folded a_dst vector.
  - Pad slots point at a dedicated pad row whose x column is chosen so
    that alpha_src(pad) = -60 and alpha_dst(pad) = 0, which makes exp()
    underflow to exactly 0 (no contribution).
  - Softmax over slots runs on DVE/ACT per head; the gathered rows are
    scaled in place by the unnormalized attention and summed on the tensor
    engine (identity-weight accumulating matmuls into PSUM), then
    normalized by the softmax denominator.
  - The layer-2 table row is [h2(64) | alpha_src2 | alpha_dst2 | pad] in
    128 bf16 columns (256B).  It is built from relu(out1) via a PE
    transpose + matmul, all-gathered across the 8 cores, and layer 2
    repeats the same gather/softmax/weighted-sum with 1 head.
  - Host does only integer graph partitioning (permutation, window/slot
    assignment, index arrays) and the final inverse permutation.
"""

import math

import numpy as np

# ---- problem constants (test code may override these before calling kernel) ----
N = 50000
E = 1600000
IN_CH = 128
HEADS = 4
MID = 32
OUT_CH = 64
NEG_SLOPE = 0.2
N_CORES = 8
P = 128

F1 = 256                  # layer-1 table row BYTES: [h fp8(128)|as bf16(4)|pad]
F2 = 128                  # layer-2 table row: [h2(64)|as2|ad2|pad] (bf16)
WIN = 32768               # int16-addressable window (rows)
PADB = 64.0               # pad logit: exp(leaky(-PADB)) ~ 2e-6; h_pad must
                          # stay under the fp8-e4m3 max (240)

_cache = {}


def _skips():
    return set(s for s in _SKIP.split(",") if s)


_REPS = 1                 # debug: repeat phases B..C (idempotent) to amplify
                          # device time above the measurement noise floor
_SKIP = ""                # debug: comma-separated of B,C,AG,V,W,P,G,D,S


def _ranks_within(key):
    """Rank of each element among equal keys (stable, 0-based)."""
    n = len(key)
    if n == 0:
        return np.zeros(0, np.int64)
    o = np.argsort(key, kind="stable")
    ks = key[o]
    new = np.r_[True, ks[1:] != ks[:-1]]
    starts = np.nonzero(new)[0]
    lens = np.diff(np.r_[starts, n])
    r = np.arange(n) - np.repeat(starts, lens)
    out = np.empty(n, np.int64)
    out[o] = r
    return out


def _host_prep(x, edge_index):
    n_own = N // N_CORES
    assert N % N_CORES == 0
    K = math.ceil(n_own / P)
    ppc = K * P
    n_pad = ppc - n_own
    n_pos = ppc * N_CORES
    assert n_pad >= 2, "need at least two pad rows per core"

    src = np.asarray(edge_index[0], dtype=np.int64)
    dst = np.asarray(edge_index[1], dtype=np.int64)
    loops = np.arange(N, dtype=np.int64)
    src = np.concatenate([src, loops])
    dst = np.concatenate([dst, loops])

    deg = np.bincount(dst, minlength=N)
    core_of = np.arange(N) // n_own

    order = np.lexsort((-deg, core_of))
    pos_of_node = np.empty(N, np.int64)
    node_at_pos = np.full(n_pos, -1, np.int64)
    for c in range(N_CORES):
        nodes = order[c * n_own:(c + 1) * n_own]
        p0 = c * ppc
        pos_of_node[nodes] = p0 + np.arange(n_own)
        node_at_pos[p0:p0 + n_own] = nodes

    win = min(WIN, n_pos)
    ps = pos_of_node[src]
    pd = pos_of_node[dst]
    c_arr = pd // ppc
    rem = pd % ppc
    k_arr = rem // P
    p_arr = rem % P

    low = ps < win
    par = (ps & 1).astype(np.int64)
    mLE = low & (par == 0)
    mLO = low & (par == 1)
    mHE = (~low) & (par == 0)
    mHO = (~low) & (par == 1)

    def rowcount(m):
        cnt = np.zeros(n_pos, np.int64)
        np.add.at(cnt, pd[m], 1)
        return cnt

    cHE = rowcount(mHE)
    cHO = rowcount(mHO)
    cLE = rowcount(mLE)
    cLO = rowcount(mLO)
    d_row = cHE + cHO + cLE + cLO

    def chunkmax(a):
        return a.reshape(N_CORES, K, P).max(axis=(0, 2))

    mBe = chunkmax(cHE)
    mCo = chunkmax(cHO)
    maxd = chunkmax(d_row)
    # A must absorb: total overflow (d - TB - TC) and per-parity overflow
    # (even_count - TB, odd_count - TC), per chunk.
    TA = np.maximum.reduce([
        maxd - mBe - mCo,
        chunkmax(cHE + cLE) - mBe,
        chunkmax(cHO + cLO) - mCo,
        np.zeros(K, np.int64),
    ])

    # Per-row: keep B at <= mBe and C at <= mCo; push the excess lows to A.
    kpos = (np.arange(n_pos) % ppc) // P
    aE = np.maximum(0, cHE + cLE - mBe[kpos])
    aO = np.maximum(0, cHO + cLO - mCo[kpos])
    B_load = cHE + (cLE - aE)
    C_load = cHO + (cLO - aO)
    TB = chunkmax(B_load)
    TC = chunkmax(C_load)
    T_k = TA + TB + TC
    S = int(T_k.sum())

    # per-edge ranks within (row, class)
    col = np.empty(len(src), np.int64)
    winid = np.empty(len(src), np.int64)  # 0=A 1=B 2=C

    rk_he = _ranks_within(pd[mHE])
    rk_ho = _ranks_within(pd[mHO])
    rk_le = _ranks_within(pd[mLE])
    rk_lo = _ranks_within(pd[mLO])

    # forced high classes
    winid[mHE] = 1
    col[mHE] = rk_he
    winid[mHO] = 2
    col[mHO] = rk_ho
    # low-even: A while rank < aE, else overflow to B after the forced ones
    rows_le = pd[mLE]
    toA = rk_le < aE[rows_le]
    wle = np.where(toA, 0, 1)
    cle = np.where(toA, rk_le, cHE[rows_le] + (rk_le - aE[rows_le]))
    winid[mLE] = wle
    col[mLE] = cle
    # low-odd: A (after the evens) while rank < aO, else overflow to C
    rows_lo = pd[mLO]
    toA = rk_lo < aO[rows_lo]
    wlo = np.where(toA, 0, 2)
    clo = np.where(toA, aE[rows_lo] + rk_lo,
                   cHO[rows_lo] + (rk_lo - aO[rows_lo]))
    winid[mLO] = wlo
    col[mLO] = clo

    # idx tile layout: per chunk k the blocks [A | B | C], block w has
    # 8*T_w(k) int16 columns (wrapped in 16 partitions).
    blk = np.zeros((K, 3), np.int64)
    blk[:, 0] = TA
    blk[:, 1] = TB
    blk[:, 2] = TC
    blk_off = np.zeros((K, 3), np.int64)
    flat = blk.reshape(-1)
    off = np.cumsum(np.r_[0, flat[:-1]])
    blk_off[:, :] = off.reshape(K, 3)
    COLS = int(flat.sum()) * 8          # idx columns (16-partition wrap)

    pad_pos = n_own                      # core 0's first pad row (even)
    assert pad_pos % 2 == 0 and pad_pos + 1 < ppc
    padA = pad_pos
    padBC = pad_pos >> 1                 # row (pad_pos, pad_pos+1) pair

    idx = np.empty((N_CORES, 16, COLS), np.int16)
    for k in range(K):
        for w in range(3):
            o = blk_off[k, w] * 8
            t = blk[k, w] * 8
            idx[:, :, o:o + t] = padA if w == 0 else padBC

    # scatter real edges
    ecol = col + blk_off[k_arr, winid]   # slot column within the chunk row
    j = (col * 128 + p_arr)              # index position within its gather
    part = j % 16
    colidx = blk_off[k_arr, winid] * 8 + (col * 8 + p_arr // 16)
    val = np.where(winid == 0, ps, ps >> 1).astype(np.int16)
    idx[c_arr, part, colidx] = val
    idx_full = np.broadcast_to(idx[:, None, :, :],
                               (N_CORES, 8, 16, COLS)).reshape(
                                   N_CORES, 128, COLS).copy()

    xT = np.zeros((IN_CH, n_pos), np.float32)
    m = node_at_pos >= 0
    xT[:, m] = np.asarray(x, np.float32)[node_at_pos[m]].T

    return dict(n_own=n_own, K=K, ppc=ppc, n_pad=n_pad, n_pos=n_pos,
                TA=tuple(int(v) for v in TA), TB=tuple(int(v) for v in TB),
                TC=tuple(int(v) for v in TC),
                blk_off=tuple(tuple(int(v) for v in r) for r in blk_off),
                S=S, COLS=COLS, idx=idx_full, xT=xT,
                node_at_pos=node_at_pos, win=win)


def _aug_weights(W1, a_src1, a_dst1, W2, a_src2, a_dst2):
    W1 = np.asarray(W1, np.float32)
    W2 = np.asarray(W2, np.float32)
    A1s = np.einsum("chm,hm->ch", W1.reshape(IN_CH, HEADS, MID),
                    np.asarray(a_src1, np.float32))
    A1d = np.einsum("chm,hm->ch", W1.reshape(IN_CH, HEADS, MID),
                    np.asarray(a_dst1, np.float32))
    A2s = W2 @ np.asarray(a_src2, np.float32).reshape(OUT_CH, 1)
    A2d = W2 @ np.asarray(a_dst2, np.float32).reshape(OUT_CH, 1)
    W2a = np.concatenate([W2, A2s, A2d], axis=1)          # [128, 66]
    return A1s, A1d, W2a


def _build_program(K, TA, TB, TC, blk_off, S, COLS, n_pos, ppc, n_own,
                   n_pad, win):
    import concourse.bass as bass
    import concourse.mybir as mybir
    import concourse.tile as tile
    from concourse import library_config
    from concourse.library_overlay import lower_extended_insts
    from concourse.masks import make_identity

    f32 = mybir.dt.float32
    bf16 = mybir.dt.bfloat16
    i16 = mybir.dt.int16
    i8 = mybir.dt.int8
    fp8 = mybir.dt.float8e4
    FT2A = OUT_CH + 2                # 66 real columns of a T2 row
    Alu = mybir.AluOpType
    Act = mybir.ActivationFunctionType
    Ax = mybir.AxisListType

    W1A = IN_CH + HEADS              # [W1 | A1s] columns
    C_W1 = 0
    C_A1D = C_W1 + W1A
    C_W2A = C_A1D + HEADS
    C_B1 = C_W2A + FT2A
    C_B2 = C_B1 + IN_CH
    C_PAD2 = C_B2 + OUT_CH           # [P, 1]: 0 on real rows, -PADB on pads
    C_END = C_PAD2 + 1

    nc = bass.Bass(num_swdge_queues=4)
    xTP = nc.declare_dram_parameter("xT", [IN_CH, n_pos], f32, isOutput=False)
    xToP = nc.declare_dram_parameter("xTo", [IN_CH, ppc], f32, isOutput=False)
    constP = nc.declare_dram_parameter("consts", [P, C_END], f32,
                                       isOutput=False)
    idxP = nc.declare_dram_parameter("idx16", [P, COLS], i16, isOutput=False)
    outP = nc.declare_dram_parameter("out", [ppc, OUT_CH], f32, isOutput=True)

    T1 = nc.dram_tensor("T1", [n_pos, F1], i8)
    T2s = nc.dram_tensor("T2s", [ppc, F2], bf16)
    T2 = nc.dram_tensor("T2", [n_pos, F2], bf16, addr_space="Shared")
    T2L = nc.dram_tensor("T2L", [n_pos, F2], bf16)

    n_tiles = n_pos // P

    with tile.TileContext(nc) as tc:
        nc.gpsimd.load_library(library_config.mlp)
        with tc.tile_pool(name="const", bufs=1) as cpool:
            consts = cpool.tile([P, C_END], f32)
            nc.sync.dma_start(out=consts[:], in_=constP[:, :])
            w1a_t = consts[:, C_W1:C_W1 + W1A]
            a1d_t = consts[:, C_A1D:C_A1D + HEADS]
            w2a_t = consts[:, C_W2A:C_W2A + FT2A]
            b1r_t = consts[:, C_B1:C_B1 + IN_CH]
            b2r_t = consts[:, C_B2:C_B2 + OUT_CH]
            pad2_t = consts[:, C_PAD2:C_PAD2 + 1]
            idx_t = cpool.tile([P, COLS], i16)
            nc.sync.dma_start(out=idx_t[:], in_=idxP[:, :])

            ident_b = cpool.tile([P, P], bf16)
            make_identity(nc, ident_b[:])
            t2stage = cpool.tile([P, K * F2], bf16)
            nc.vector.memset(t2stage[:], 0)
            ad_t = cpool.tile([P, K * HEADS], f32)

            # num_idxs registers, one per distinct gather size
            sizes = sorted({128 * t for t in (*TA, *TB, *TC) if t > 0})
            regs = {s: nc.gpsimd.to_reg(s) for s in sizes}

            # ---------------- phase A: build T1 + own alpha_dst ----------
            NSPLIT = max(1, min(8, n_tiles // 8))
            bounds = [n_tiles * i // NSPLIT for i in range(NSPLIT + 1)]
            with tc.tile_pool(name="pa_x", bufs=3) as xpool, \
                 tc.tile_pool(name="pa_st", bufs=2) as stpool, \
                 tc.tile_pool(name="pa_ps", bufs=2, space="PSUM") as pspool:
                GRP = 8
                for s_ in range(NSPLIT):
                    lo, hi = bounds[s_], bounds[s_ + 1]
                    stg = stpool.tile([P, (hi - lo) * F1], i8, tag="stg")
                    nc.vector.memset(stg[:], 0)
                    stg3 = stg[:].rearrange("p (t f) -> p t f", f=F1)
                    for t0 in range(lo, hi, GRP):
                        g = min(GRP, hi - t0)
                        xt = xpool.tile([IN_CH, g * P], f32, tag="xt")
                        nc.sync.dma_start(out=xt[:],
                                          in_=xTP[:, t0 * P:(t0 + g) * P])
                        for j in range(g):
                            ps = pspool.tile([P, W1A], f32, tag="ps")
                            nc.tensor.matmul(ps[:],
                                             lhsT=xt[:, j * P:(j + 1) * P],
                                             rhs=w1a_t, start=True,
                                             stop=True)
                            t = t0 + j - lo
                            nc.scalar.copy(
                                stg3[:, t, 0:IN_CH].bitcast(fp8),
                                ps[:, 0:IN_CH])
                            nc.scalar.copy(
                                stg3[:, t, IN_CH:IN_CH + 2 * HEADS]
                                .bitcast(bf16),
                                ps[:, IN_CH:IN_CH + HEADS])
                    dview = T1[lo * P:hi * P, :].rearrange(
                        "(t p) f -> p t f", p=P)
                    nc.sync.dma_start(
                        out=dview,
                        in_=stg[:].rearrange("p (t f) -> p t f", f=F1))
                # own alpha_dst: [P, 4] per chunk
                xo = xpool.tile([IN_CH, ppc], f32, tag="xo")
                nc.sync.dma_start(out=xo[:], in_=xToP[:, :])
                for k in range(K):
                    psd = pspool.tile([P, HEADS], f32, tag="psd")
                    nc.tensor.matmul(psd[:], lhsT=xo[:, k * P:(k + 1) * P],
                                     rhs=a1d_t, start=True, stop=True)
                    nc.vector.tensor_copy(
                        ad_t[:, k * HEADS:(k + 1) * HEADS], psd[:])

            T1A = T1[0:win, :]
            T1v = T1[:, :].rearrange("(a b) f -> a b f", b=2)
            T2A = T2L[0:win, :]
            T2v = T2L[:, :].rearrange("(a b) f -> a b f", b=2)

            qctr = [0]

            def gathers(k, G3, tabA, tabB, tabC, F):
                """Issue the up-to-3 window gathers for chunk k into G3."""
                if "D" in _skips():
                    return
                ta, tb, tcn = TA[k], TB[k], TC[k]
                o = 0
                for w, (t, tab, step) in enumerate((
                        (ta, tabA, None), (tb, tabB, 2 * F),
                        (tcn, tabC, 2 * F))):
                    if t == 0:
                        continue
                    co = blk_off[k][w] * 8
                    nc.gpsimd.dma_gather(
                        G3[:, o:o + t], tab,
                        idx_t[:, co:co + 8 * t], 128 * t, regs[128 * t],
                        F, elem_step=step, single_packet=(128 * t <= 1024),
                        queue_num=qctr[0] & 3)
                    qctr[0] += 1
                    o += t

            for _rep in range(_REPS):
              # ---------------- phase B: layer-1 aggregation ------------
              if "B" not in _skips():
                with tc.tile_pool(name="pb_g", bufs=4) as gpool, \
                     tc.tile_pool(name="pb_t", bufs=3) as tpool, \
                     tc.tile_pool(name="pb_sm", bufs=4) as smpool, \
                     tc.tile_pool(name="pb_ps", bufs=2, space="PSUM") as psB, \
                     tc.tile_pool(name="pb_pst", bufs=2, space="PSUM") as psT, \
                     tc.tile_pool(name="pb_psu", bufs=2, space="PSUM") as psU:
                    for k in range(K):
                        T = TA[k] + TB[k] + TC[k]
                        G = gpool.tile([P, T * F1], i8, tag="G")
                        G3 = G[:].rearrange("p (d f) -> p d f", f=F1)
                        gathers(k, G3, T1A, T1v[:, 0], T1v[:, 1], F1)

                        if "G" in _skips():
                            continue
                        # logits: gathered alpha_src (bf16 in-row) + own
                        # alpha_dst, then leaky-relu
                        logits = smpool.tile([P, HEADS * T], f32,
                                             tag="logits")
                        for h in range(HEADS):
                            lh = logits[:, h * T:(h + 1) * T]
                            asrc_h = G3[:, :, IN_CH + 2 * h:
                                        IN_CH + 2 * h + 2] \
                                .bitcast(bf16).squeeze(2)
                            nc.vector.tensor_scalar_add(
                                lh, asrc_h, ad_t[:, k * HEADS + h:
                                                 k * HEADS + h + 1])
                            nc.vector.scalar_tensor_tensor(
                                lh, lh, NEG_SLOPE, lh, op0=Alu.mult,
                                op1=Alu.max)
                        e_t = smpool.tile([P, HEADS * T], f32, tag="e")
                        s_t = smpool.tile([P, HEADS], f32, tag="s")
                        for h in range(HEADS):
                            nc.scalar.activation(
                                e_t[:, h * T:(h + 1) * T],
                                logits[:, h * T:(h + 1) * T],
                                Act.Exp,
                                accum_out=s_t[:, h:h + 1])
                        nc.vector.tensor_scalar_add(s_t[:], s_t[:], 1e-30)
                        rcp = smpool.tile([P, HEADS], f32, tag="rcp")
                        nc.vector.reciprocal(rcp[:], s_t[:])

                        # scale gathered h by unnormalized attention
                        sc = tpool.tile([P, T * IN_CH], bf16, tag="sc")
                        sc4 = sc[:].rearrange("p (d h c) -> p d h c",
                                              h=HEADS, c=MID)
                        hview = G3[:, :, 0:IN_CH].bitcast(fp8) \
                            .rearrange("p d (h c) -> p d h c", c=MID)
                        e_b = e_t[:].rearrange("p (h d) -> p d h", d=T) \
                            .unsqueeze(3).to_broadcast([P, T, HEADS, MID])
                        nc.vector.tensor_tensor(out=sc4, in0=hview,
                                                in1=e_b, op=Alu.mult)

                        ps = psB.tile([P, IN_CH], f32, tag="acc")
                        TP = 1 if "P" in _skips() else T
                        sc3 = sc[:].rearrange("p (d f) -> p d f", f=IN_CH)
                        for d in range(TP):
                            nc.tensor.matmul(ps[:], lhsT=ident_b[:],
                                             rhs=sc3[:, d],
                                             start=(d == 0),
                                             stop=(d == TP - 1))

                        tmp2 = smpool.tile([P, IN_CH], f32, tag="tmp2")
                        rcp_b = rcp[:].unsqueeze(2).to_broadcast(
                            [P, HEADS, MID])
                        nc.vector.tensor_tensor(
                            out=tmp2[:].rearrange("p (h c) -> p h c", c=MID),
                            in0=ps[:].rearrange("p (h c) -> p h c", c=MID),
                            in1=rcp_b, op=Alu.mult)
                        nc.vector.tensor_add(tmp2[:], tmp2[:], b1r_t)
                        r1 = smpool.tile([P, IN_CH], bf16, tag="r1")
                        nc.scalar.activation(r1[:], tmp2[:], Act.Relu)

                        tps = psT.tile([P, P], bf16, tag="tps")
                        nc.tensor.transpose(tps[:], r1[:], ident_b[:])
                        r1T = smpool.tile([P, P], f32, tag="r1T")
                        nc.vector.tensor_copy(r1T[:], tps[:])
                        t2ps = psU.tile([P, FT2A], f32, tag="t2ps")
                        nc.tensor.matmul(t2ps[:], lhsT=r1T[:], rhs=w2a_t,
                                         start=True, stop=True)
                        nc.scalar.copy(
                            t2stage[:, k * F2:k * F2 + FT2A], t2ps[:])
                        # own pad rows (last chunk): alpha_src2 += -PADB
                        if k == K - 1 and n_pad:
                            nc.vector.tensor_add(
                                t2stage[:, k * F2 + OUT_CH:
                                        k * F2 + OUT_CH + 1],
                                t2ps[:, OUT_CH:OUT_CH + 1],
                                pad2_t)

              if True:
                nc.sync.dma_start(
                    out=T2s[:, :].rearrange("(k p) f -> p k f", p=P),
                    in_=t2stage[:].rearrange("p (k f) -> p k f", f=F2))

              if "AG" not in _skips():
                nc.gpsimd.collective_compute(
                    "AllGather",
                    mybir.AluOpType.bypass,
                    replica_groups=[list(range(N_CORES))],
                    ins=[T2s[:, :]],
                    outs=[T2[:, :]],
                )
              if True:
                # stage the shared table into local DRAM: random 256B reads
                # from Shared space are ~2x slower than from local HBM
                nc.sync.dma_start(
                    out=T2L[:, :].rearrange("(p q) f -> p (q f)", p=P),
                    in_=T2[:, :].rearrange("(p q) f -> p (q f)", p=P))

              # ---------------- phase C: layer-2 aggregation ------------
              if "C" not in _skips():
                with tc.tile_pool(name="pc_g", bufs=4) as g2pool, \
                     tc.tile_pool(name="pc_sm", bufs=4) as sm2pool, \
                     tc.tile_pool(name="pc_ps", bufs=2, space="PSUM") as psC:
                    for k in range(K):
                        T = TA[k] + TB[k] + TC[k]
                        G2 = g2pool.tile([P, T * F2], bf16, tag="G2")
                        G23 = G2[:].rearrange("p (d f) -> p d f", f=F2)
                        if "S" in _skips():
                            gathers(k, G23, T1A, T1v[:, 0], T1v[:, 1], F1)
                        else:
                            gathers(k, G23, T2A, T2v[:, 0], T2v[:, 1], F2)

                        if "G" in _skips():
                            continue
                        adst2 = sm2pool.tile([P, 1], f32, tag="adst2")
                        nc.vector.tensor_copy(
                            adst2[:],
                            t2stage[:, k * F2 + OUT_CH + 1:
                                    k * F2 + OUT_CH + 2])

                        logits2 = sm2pool.tile([P, T], f32, tag="logits2")
                        asrc2 = G23[:, :, OUT_CH:OUT_CH + 1].squeeze(2)
                        nc.vector.tensor_scalar_add(logits2[:], asrc2,
                                                    adst2[:, 0:1])
                        nc.vector.scalar_tensor_tensor(
                            logits2[:], logits2[:], NEG_SLOPE, logits2[:],
                            op0=Alu.mult, op1=Alu.max)
                        e2 = sm2pool.tile([P, T], f32, tag="e2")
                        s2 = sm2pool.tile([P, 1], f32, tag="s2")
                        nc.scalar.activation(e2[:], logits2[:], Act.Exp,
                                             accum_out=s2[:, 0:1])
                        nc.vector.tensor_scalar_add(s2[:], s2[:], 1e-30)
                        rcp2 = sm2pool.tile([P, 1], f32, tag="rcp2")
                        nc.vector.reciprocal(rcp2[:], s2[:])

                        if "W" not in _skips():
                            h2view = G23[:, :, 0:OUT_CH]
                            e2_b = e2[:].unsqueeze(2).to_broadcast(
                                [P, T, OUT_CH])
                            nc.vector.tensor_tensor(out=h2view, in0=h2view,
                                                    in1=e2_b, op=Alu.mult)

                        ps2 = psC.tile([P, OUT_CH], f32, tag="acc2")
                        TP = 1 if "P" in _skips() else T
                        for d in range(TP):
                            nc.tensor.matmul(ps2[:], lhsT=ident_b[:],
                                             rhs=G23[:, d, 0:OUT_CH],
                                             start=(d == 0),
                                             stop=(d == TP - 1))

                        outt = sm2pool.tile([P, OUT_CH], f32, tag="outt")
                        nc.scalar.activation(outt[:], ps2[:], Act.Identity,
                                             scale=rcp2[:, 0:1])
                        nc.vector.tensor_add(outt[:], outt[:], b2r_t)
                        nc.sync.dma_start(out=outP[k * P:(k + 1) * P, :],
                                          in_=outt[:])

    lower_extended_insts(nc)
    _split_excess_waits(nc, mybir)
    return nc


def _split_excess_waits(nc, mybir):
    """Walrus allows only one sync-wait command per instruction here.
    Hoist excess waits onto freshly inserted same-engine NoOps (safe:
    waiting earlier on the same engine)."""
    ctr = 0
    for bb in nc.main_func.blocks:
        out = []
        changed = False
        for ins in bb.instructions:
            si = ins.sync_info
            waits = list(si.on_wait) if (si is not None and si.on_wait) else []
            if len(waits) > 1:
                keep = waits[-1:]
                excess = waits[:-1]
                for w in excess:
                    ctr += 1
                    nop = mybir.InstNoOp(
                        name=f"waitsplit-{ctr}-{ins.name}",
                        opcode="NoOp",
                        engine=ins.engine,
                        sync_info=mybir.SyncInfo(on_wait=[w], on_update=[]),
                    )
                    out.append(nop)
                ins.sync_info = mybir.SyncInfo(
                    on_wait=keep,
                    on_update=list(si.on_update) if si.on_update else [])
                changed = True
            out.append(ins)
        if changed:
            try:
                bb.instructions[:] = out
            except TypeError:
                bb.instructions = out


def _make_runner(nc, n_cores):
    import jax
    from jax.sharding import Mesh, PartitionSpec
    from jax.experimental.shard_map import shard_map
    from concourse import bass2jax
    import concourse.mybir as mybir

    bass2jax.install_neuronx_cc_hook()
    partition_name = (nc.partition_id_tensor.name
                      if nc.partition_id_tensor else None)
    in_names = []
    out_names = []
    out_avals = []
    zero_outs = []
    for alloc in nc.m.functions[0].allocations:
        if not isinstance(alloc, mybir.MemoryLocationSet):
            continue
        name = alloc.memorylocations[0].name
        if alloc.kind == "ExternalInput":
            if name != partition_name:
                in_names.append(name)
        elif alloc.kind == "ExternalOutput":
            shape = tuple(alloc.tensor_shape)
            dtype = mybir.dt.np(alloc.dtype)
            out_names.append(name)
            out_avals.append(jax.core.ShapedArray(shape, dtype))
            zero_outs.append(np.zeros(shape, dtype))
    n_params = len(in_names)
    all_names = list(in_names) + out_names
    if partition_name is not None:
        all_names.append(partition_name)

    def _body(*args):
        operands = list(args)
        if partition_name is not None:
            operands.append(bass2jax.partition_id_tensor())
        outs = bass2jax._bass_exec_p.bind(
            *operands,
            out_avals=tuple(out_avals),
            in_names=tuple(all_names),
            out_names=tuple(out_names),
            lowering_input_output_aliases=(),
            sim_require_finite=True,
            sim_require_nnan=True,
            nc=nc,
        )
        return tuple(outs)

    devices = jax.devices()[:n_cores]
    mesh = Mesh(np.asarray(devices), ("core",))
    nio = n_params + len(out_names)
    sharded = jax.jit(
        shard_map(_body, mesh=mesh, in_specs=(PartitionSpec("core"),) * nio,
                  out_specs=(PartitionSpec("core"),) * len(out_names),
                  check_rep=False),
        keep_unused=True,
    )
    return dict(fn=sharded, in_names=in_names, out_names=out_names,
                zero_outs=zero_outs, mesh=mesh, n_cores=n_cores)


def _execute(runner, in_maps):
    import jax
    n_cores = runner["n_cores"]
    concat_in = [
        np.concatenate([np.asarray(in_maps[c][name])
                        for c in range(n_cores)], axis=0)
        for name in runner["in_names"]
    ]
    concat_zeros = [
        np.zeros((n_cores * z.shape[0], *z.shape[1:]), z.dtype)
        for z in runner["zero_outs"]
    ]
    out_arrs = runner["fn"](*concat_in, *concat_zeros)
    out_arrs = [np.asarray(a) for a in out_arrs]
    res = []
    for c in range(n_cores):
        m = {}
        for i, name in enumerate(runner["out_names"]):
            a = out_arrs[i]
            s0 = a.shape[0] // n_cores
            m[name] = a[c * s0:(c + 1) * s0]
        res.append(m)
    return res


def _time_exec(runner, in_maps, iters=5):
    """Steady-state wall-clock of the compiled NEFF execution (device-resident
    inputs, no host transfers in the loop)."""
    import time as _time

    import jax
    from jax.sharding import NamedSharding, PartitionSpec

    n_cores = runner["n_cores"]
    sh = NamedSharding(runner["mesh"], PartitionSpec("core"))
    concat_in = [
        np.concatenate([np.asarray(in_maps[c][name])
                        for c in range(n_cores)], axis=0)
        for name in runner["in_names"]
    ]
    concat_zeros = [
        np.zeros((n_cores * z.shape[0], *z.shape[1:]), z.dtype)
        for z in runner["zero_outs"]
    ]
    dev_in = [jax.device_put(a, sh) for a in concat_in]
    dev_z = [jax.device_put(a, sh) for a in concat_zeros]
    times = []
    for _ in range(iters):
        t0 = _time.perf_counter()
        outs = runner["fn"](*dev_in, *dev_z)
        for o in outs:
            o.block_until_ready()
        times.append(_time.perf_counter() - t0)
    return min(times), times


def _get_compiled(inputs):
    x = np.asarray(inputs["x"], np.float32)
    prep = _host_prep(x, np.asarray(inputs["edge_index"]))
    key = (prep["K"], prep["TA"], prep["TB"], prep["TC"], prep["n_pos"],
           prep["ppc"], prep["n_own"], prep["n_pad"], prep["win"], _REPS, _SKIP)
    if key not in _cache:
        nc = _build_program(prep["K"], prep["TA"], prep["TB"], prep["TC"],
                            prep["blk_off"], prep["S"], prep["COLS"],
                            prep["n_pos"], prep["ppc"], prep["n_own"],
                            prep["n_pad"], prep["win"])
        _cache[key] = _make_runner(nc, N_CORES)
    runner = _cache[key]

    A1s, A1d, W2a = _aug_weights(inputs["W1"], inputs["a_src1"],
                                 inputs["a_dst1"], inputs["W2"],
                                 inputs["a_src2"], inputs["a_dst2"])
    W1 = np.asarray(inputs["W1"], np.float32)
    b1r = np.broadcast_to(np.asarray(inputs["b1"], np.float32),
                          (P, IN_CH)).copy()
    b2r = np.broadcast_to(np.asarray(inputs["b2"], np.float32),
                          (P, OUT_CH)).copy()
    FT2A = OUT_CH + 2
    W1A = IN_CH + HEADS
    C_END = W1A + HEADS + FT2A + IN_CH + OUT_CH + 1
    consts = np.zeros((P, C_END), np.float32)
    o = 0
    consts[:IN_CH, o:o + IN_CH] = W1
    consts[:IN_CH, o + IN_CH:o + W1A] = A1s
    o += W1A
    consts[:IN_CH, o:o + HEADS] = A1d
    o += HEADS
    consts[:IN_CH, o:o + FT2A] = W2a
    o += FT2A
    consts[:, o:o + IN_CH] = b1r
    o += IN_CH
    consts[:, o:o + OUT_CH] = b2r
    o += OUT_CH
    # additive pad mask for layer-2 alpha_src of own pad rows
    padrow = np.zeros(P, np.float32)
    r = prep["n_own"] % P
    if prep["n_pad"]:
        padrow[r:] = -PADB
    consts[:, o:o + 1] = padrow[:, None]

    # pad-row x: alpha_src(pad) = -PADB for all heads, alpha_dst(pad) = 0
    Amat = np.concatenate([A1s, A1d], axis=1)     # [128, 8]
    b = np.concatenate([np.full(HEADS, -PADB, np.float32),
                        np.zeros(HEADS, np.float32)])
    x_pad, *_ = np.linalg.lstsq(Amat.T, b, rcond=None)
    h_pad = W1.T @ x_pad
    assert np.abs(h_pad).max() < 230.0, (
        f"pad row overflows fp8 range: {np.abs(h_pad).max():.1f}")
    xT = prep["xT"].copy()
    n_own, ppc = prep["n_own"], prep["ppc"]
    for c in range(N_CORES):
        xT[:, c * ppc + n_own:(c + 1) * ppc] = x_pad[:, None]

    in_maps = []
    for c in range(N_CORES):
        in_maps.append({
            "xT": xT,
            "xTo": np.ascontiguousarray(xT[:, c * ppc:(c + 1) * ppc]),
            "consts": consts,
            "idx16": prep["idx"][c],
        })
    return runner, in_maps, prep


def _run(inputs):
    runner, in_maps, prep = _get_compiled(inputs)
    # transient NRT_EXEC_UNIT_UNRECOVERABLE hiccups have been observed on
    # this runtime; back off and retry a couple of times
    import time as _time
    last_exc = None
    for attempt in range(3):
        try:
            results = _execute(runner, in_maps)
            break
        except Exception as exc:
            last_exc = exc
            _time.sleep(2.0 + 4.0 * attempt)
    else:
        raise last_exc
    out = np.empty((N, OUT_CH), np.float32)
    n_own, ppc = prep["n_own"], prep["ppc"]
    for c in range(N_CORES):
        o = np.asarray(results[c]["out"])
        nodes = prep["node_at_pos"][c * ppc:c * ppc + n_own]
        out[nodes] = o[:n_own]
    return out


def kernel(**inputs):
    return _run(inputs)
